# revision 1
# baseline (speedup 1.0000x reference)
"""Trainium2 Bass kernel for nn_MultiHeadSTEVESA.

Strategy: data-parallel over batch (8 elems per core, 8 cores).
Per-element pipeline on device; activations feature-on-partition (CT layout
[C_part, N_free]) so weights are the stationary matmul operand and
activations stream. LayerNorms are folded into the following matmul
(host-fused gamma into W, beta into bias, mean term via a rank-1 K=1 PSUM
accumulation, rstd applied at PSUM evacuation with a PE-broadcast chunk).
Phase A (pos+LN+MLP+LN+K/V) is chunked over 512-token slices end-to-end;
only k (bf16) and v^T (bf16, with a fused ones-column) stay resident.
Attention runs token-on-partition (logits^T via k-chunks as lhsT) so the
joint softmax is a native free-axis reduction; the renorm denominator is
the ones-column of v^T, fused into the update matmul.
Big matmuls use float32r (FP22 multiply, fp32 accumulate); logits/update
use bf16 operands.
"""

import os
import sys

import numpy as np

sys.path.insert(0, "/opt/trn_rl_repo")

import concourse.bass as bass
import concourse.mybir as mybir
import concourse.tile as tile
from concourse import bacc, bass_utils
from concourse.alu_op_type import AluOpType
from concourse.masks import make_identity

AF = mybir.ActivationFunctionType
AX = mybir.AxisListType
f32 = mybir.dt.float32
f32r = mybir.dt.float32r
bf16 = mybir.dt.bfloat16
ts = bass.ts

# Problem shapes
B, C, RES = 64, 256, 64
S, SLOT, H, MLP_H, OUT = 24, 256, 4, 1024, 256
ITERS = 3
EPS = 1e-8
LN_EPS = 1e-5
DH = SLOT // H

P = 128
KC = C // P            # 2 feature chunks
N = RES * RES          # 4096 tokens
NCH = 512              # token chunk for phase A
NB = N // NCH          # 8
NL = N // P            # 32 token chunks for attention
HSP = 128              # padded (head, slot) dim: hs' = h*32 + s
GC = 3 * SLOT // P     # 6 GRU gate chunks
MC_MLP = MLP_H // P    # 8
VW = 260               # vT tile width: 256 v-cols + 1 ones col + pad
NCORES = 8
BP = B // NCORES       # 8 batch elems per core


def _build_program(bp=BP):
    nc = bacc.Bacc(
        "TRN2",
        target_bir_lowering=False,
        debug=False,
        enable_asserts=False,
        num_devices=NCORES,
    )

    # ---- DRAM I/O ----
    d = {}

    def din(name, shape, dt=f32):
        d[name] = nc.dram_tensor(name, shape, dt, kind="ExternalInput").ap()
        return d[name]

    xin = din("xin", [bp, KC, P, N], f32r)
    din("w1t", [P, KC, C], f32r)
    din("r1k", [1, C], f32r)
    din("c1c", [P, KC])
    din("w2t", [P, KC, C], f32r)
    din("b2c", [P, KC])
    din("wkt", [P, KC, C], f32r)
    din("rkk", [1, C], f32r)
    din("ckc", [P, KC])
    din("wvt", [P, KC, C], f32r)
    din("rvk", [1, C], f32r)
    din("cvc", [P, KC])
    din("wqt", [P, KC, C], f32r)
    din("rqk", [1, C], f32r)
    din("cqc", [P, KC])
    din("wit", [P, KC, 3 * SLOT], f32r)
    din("wht", [P, KC, 3 * SLOT], f32r)
    din("brz", [P, 4])
    din("bhn", [P, KC])
    din("bin", [P, KC])
    din("m1t", [P, KC, MLP_H], f32r)
    din("r1m", [1, MLP_H], f32r)
    din("c1m", [P, MC_MLP])
    din("m2t", [P, MC_MLP, C], f32r)
    din("b2m", [P, KC])
    din("wot", [P, KC, OUT])
    din("ro", [1, OUT])
    din("co", [1, OUT])
    din("smu", [P, KC, S], f32r)

    out_d = nc.dram_tensor("out", [bp, S, OUT], f32, kind="ExternalOutput").ap()

    from contextlib import ExitStack

    with tile.TileContext(nc) as tc, ExitStack() as ctx:
        wp = ctx.enter_context(tc.tile_pool(name="wp", bufs=1))
        big = ctx.enter_context(tc.tile_pool(name="big", bufs=1))
        ch = ctx.enter_context(tc.tile_pool(name="ch", bufs=2))
        t5 = ctx.enter_context(tc.tile_pool(name="t5", bufs=2))
        rw = ctx.enter_context(tc.tile_pool(name="rw", bufs=2))
        sm = ctx.enter_context(tc.tile_pool(name="sm", bufs=3))
        slp = ctx.enter_context(tc.tile_pool(name="slp", bufs=3))
        ps = ctx.enter_context(tc.tile_pool(name="ps", bufs=8, space="PSUM"))

        def pst(shape):
            return ps.tile(shape, f32, tag="ps", name="ps")

        # ---- persistent constants / weights ----
        ident = wp.tile([P, P], f32, tag="ident")
        make_identity(nc, ident[:])
        ones_f = wp.tile([P, P], f32, tag="ones_f")
        nc.vector.memset(ones_f[:], 1.0)
        ones_r = wp.tile([P, P], f32r, tag="ones_r")
        nc.scalar.activation(ones_r[:], ones_f[:], AF.Copy)
        eps_col = wp.tile([P, 1], f32, tag="eps_col")
        nc.vector.memset(eps_col[:], LN_EPS)

        W = {}
        for name, ap in d.items():
            if name == "xin":
                continue
            t = wp.tile(list(ap.shape), ap.dtype, tag=name)
            nc.sync.dma_start(t[:], ap)
            W[name] = t

        coutb = wp.tile([S, OUT], f32, tag="coutb")
        nc.gpsimd.partition_broadcast(coutb[:], W["co"][:])

        # ---------- phase A helpers (per 512-token chunk) ----------
        def ln_stats_chunk(x):
            """x: [P, KC, NCH] -> (s1 [1,NCH] f32r, ivb [P,NCH] f32) tiles.

            Ones-matrix lhsT makes the PE emit the partition-sum broadcast
            to all 128 partitions, so the rstd chain runs full-width and no
            separate broadcast is needed."""
            xs = t5.tile([P, NCH], f32r, tag="xs")
            nc.vector.tensor_add(xs[:], x[:, 0, :], x[:, 1, :])
            p1 = pst([P, NCH])
            nc.tensor.matmul(p1[:], ones_r[:], xs[:], start=True, stop=True)
            s1 = rw.tile([1, NCH], f32r, tag="s1c")
            nc.scalar.activation(s1[:], p1[0:1, :], AF.Copy)
            q1 = t5.tile([P, NCH], f32r, tag="sq0")
            nc.scalar.activation(q1[:], x[:, 0, :], AF.Square)
            q2 = t5.tile([P, NCH], f32r, tag="sq1")
            nc.scalar.activation(q2[:], x[:, 1, :], AF.Square)
            nc.vector.tensor_add(q1[:], q1[:], q2[:])
            p2 = pst([P, NCH])
            nc.tensor.matmul(p2[:], ones_r[:], q1[:], start=True, stop=True)
            sqm = t5.tile([P, NCH], f32, tag="sqmc")
            nc.scalar.activation(sqm[:], p1[:], AF.Square, scale=1.0 / 16.0)
            nc.vector.tensor_tensor(sqm[:], p2[:], sqm[:], AluOpType.subtract)
            sd = t5.tile([P, NCH], f32, tag="sdc")
            nc.scalar.activation(
                sd[:], sqm[:], AF.Sqrt, bias=eps_col[:], scale=1.0 / C
            )
            ivb = t5.tile([P, NCH], f32, tag="ivb")
            nc.vector.reciprocal(ivb[:], sd[:])
            return s1, ivb

        def mm_layer_chunk(dst_slices, src, wt, rk, s1, ivb, bias, act):
            """dst[mc] = act(ivb*(src^T@wt - m*r)[mc] + bias[mc])."""
            for mc in range(KC):
                pu = pst([P, NCH])
                for kc in range(KC):
                    nc.tensor.matmul(
                        pu[:],
                        wt[:, kc, ts(mc, P)],
                        src[:, kc, :],
                        start=(kc == 0),
                        stop=False,
                    )
                nc.tensor.matmul(
                    pu[:], rk[:, ts(mc, P)], s1[:], start=False, stop=True
                )
                tt = t5.tile([P, NCH], f32, tag="ev")
                nc.vector.tensor_tensor(tt[:], pu[:], ivb[:], AluOpType.mult)
                nc.scalar.activation(
                    dst_slices[mc], tt[:], act, bias=bias[:, mc : mc + 1]
                )

        # ================= per batch element =================
        for e in range(bp):
            kbf = big.tile([P, KC, N], bf16, tag="kbf")
            vtt = big.tile([P, NL, VW], bf16, tag="vtt")
            nc.vector.memset(vtt[:, :, 256:257], 1.0)

            for nb in range(NB):
                sl = ts(nb, NCH)
                x0 = ch.tile([P, KC, NCH], f32r, tag="x0c")
                for kc in range(KC):
                    nc.sync.dma_start(x0[:, kc], xin[e, kc, :, sl])
                s1a, ivba = ln_stats_chunk(x0)
                h = ch.tile([P, KC, NCH], f32r, tag="hc")
                mm_layer_chunk(
                    [h[:, mc, :] for mc in range(KC)],
                    x0, W["w1t"], W["r1k"], s1a, ivba, W["c1c"], AF.Relu,
                )
                x2 = ch.tile([P, KC, NCH], f32r, tag="x2c")
                for mc in range(KC):
                    pu = pst([P, NCH])
                    for kc in range(KC):
                        nc.tensor.matmul(
                            pu[:],
                            W["w2t"][:, kc, ts(mc, P)],
                            h[:, kc, :],
                            start=(kc == 0),
                            stop=(kc == KC - 1),
                        )
                    nc.scalar.activation(
                        x2[:, mc, :], pu[:], AF.Identity,
                        bias=W["b2c"][:, mc : mc + 1],
                    )
                s1b, ivbb = ln_stats_chunk(x2)
                mm_layer_chunk(
                    [kbf[:, mc, sl] for mc in range(KC)],
                    x2, W["wkt"], W["rkk"], s1b, ivbb, W["ckc"], AF.Identity,
                )
                # v chunk, transposed into vtt on the fly
                for mc in range(KC):
                    pu = pst([P, NCH])
                    for kc in range(KC):
                        nc.tensor.matmul(
                            pu[:],
                            W["wvt"][:, kc, ts(mc, P)],
                            x2[:, kc, :],
                            start=(kc == 0),
                            stop=False,
                        )
                    nc.tensor.matmul(
                        pu[:], W["rvk"][:, ts(mc, P)], s1b[:],
                        start=False, stop=True,
                    )
                    tt = t5.tile([P, NCH], f32, tag="ev")
                    nc.vector.tensor_tensor(tt[:], pu[:], ivbb[:], AluOpType.mult)
                    vtmp = t5.tile([P, NCH], f32, tag="vtmp")
                    nc.scalar.activation(
                        vtmp[:], tt[:], AF.Identity, bias=W["cvc"][:, mc : mc + 1]
                    )
                    for j in range(NCH // P):
                        pt = pst([P, P])
                        nc.tensor.transpose(pt[:], vtmp[:, ts(j, P)], ident[:])
                        nc.scalar.activation(
                            vtt[:, nb * 4 + j, ts(mc, P)], pt[:], AF.Copy
                        )

            # ---------- slot loop ----------
            def slot_stats_row(sl_t):
                """slots [P, KC, S] -> (s1row [1,S] f32r, invb [P,S] f32)."""
                pr1 = pst([P, S])
                for kc in range(KC):
                    nc.tensor.matmul(
                        pr1[:], ones_r[:], sl_t[:, kc, :],
                        start=(kc == 0), stop=(kc == KC - 1),
                    )
                s1r = slp.tile([1, S], f32r, tag="s1r24")
                nc.scalar.activation(s1r[:], pr1[0:1, :], AF.Copy)
                pr2 = pst([P, S])
                for kc in range(KC):
                    sq = slp.tile([P, S], f32r, tag="sq24")
                    nc.scalar.activation(sq[:], sl_t[:, kc, :], AF.Square)
                    nc.tensor.matmul(
                        pr2[:], ones_r[:], sq[:],
                        start=(kc == 0), stop=(kc == KC - 1),
                    )
                sqm = slp.tile([P, S], f32, tag="sqm24")
                nc.scalar.activation(sqm[:], pr1[:], AF.Square, scale=1.0 / 16.0)
                nc.vector.tensor_tensor(sqm[:], pr2[:], sqm[:], AluOpType.subtract)
                sd = slp.tile([P, S], f32, tag="sd24")
                nc.scalar.activation(
                    sd[:], sqm[:], AF.Sqrt, bias=eps_col[:], scale=1.0 / C
                )
                invb = slp.tile([P, S], f32, tag="invb24")
                nc.vector.reciprocal(invb[:], sd[:])
                return s1r, invb

            slots = slp.tile([P, KC, S], f32r, tag="slots")
            nc.vector.tensor_copy(slots[:], W["smu"][:])

            for it in range(ITERS):
                # q projection with ln_slot folded
                s1q, invbq = slot_stats_row(slots)
                qsb = slp.tile([P, KC, S], f32, tag="qsb")
                for mc in range(KC):
                    pq = pst([P, S])
                    for kc in range(KC):
                        nc.tensor.matmul(
                            pq[:],
                            W["wqt"][:, kc, ts(mc, P)],
                            slots[:, kc, :],
                            start=(kc == 0),
                            stop=False,
                        )
                    nc.tensor.matmul(
                        pq[:], W["rqk"][:, ts(mc, P)], s1q[:],
                        start=False, stop=True,
                    )
                    tq = slp.tile([P, S], f32, tag="tq")
                    nc.vector.tensor_tensor(tq[:], pq[:], invbq[:], AluOpType.mult)
                    nc.scalar.activation(
                        qsb[:, mc, :], tq[:], AF.Identity,
                        bias=W["cqc"][:, mc : mc + 1],
                    )
                # block-diagonal Q~ (bf16), hs' = h*32 + s
                qb = slp.tile([P, KC, HSP], bf16, tag="qb")
                nc.vector.memset(qb[:], 0.0)
                for hh in range(H):
                    prange = slice((hh % 2) * 64, (hh % 2) * 64 + 64)
                    nc.vector.tensor_copy(
                        qb[prange, hh // 2, hh * 32 : hh * 32 + S],
                        qsb[prange, hh // 2, :],
                    )

                # logits^T -> exp -> b -> update (interleaved accumulation)
                psu = pst([P, SLOT + 1])
                for nl in range(NL):
                    psl = pst([P, HSP])
                    for kc in range(KC):
                        nc.tensor.matmul(
                            psl[:],
                            kbf[:, kc, ts(nl, P)],
                            qb[:, kc, :],
                            start=(kc == 0),
                            stop=(kc == KC - 1),
                        )
                    esb = sm.tile([P, HSP], f32, tag="esb")
                    nc.scalar.activation(esb[:], psl[:], AF.Exp)
                    e4 = sm.tile([P, H], f32, tag="e4")
                    ev = esb[:].rearrange("p (h s) -> p h s", s=32)
                    nc.vector.reduce_sum(e4[:], ev[:, :, 0:S], axis=AX.X)
                    trow = sm.tile([P, 1], f32, tag="trow")
                    nc.vector.reduce_sum(trow[:], e4[:], axis=AX.X)
                    rt = sm.tile([P, 1], f32, tag="rt")
                    nc.vector.reciprocal(rt[:], trow[:])
                    bch = sm.tile([P, HSP], bf16, tag="bch")
                    nc.vector.tensor_scalar(
                        bch[:], esb[:], rt[:], EPS, AluOpType.mult, AluOpType.add
                    )
                    nc.tensor.matmul(
                        psu[:],
                        bch[:],
                        vtt[:, nl, 0 : SLOT + 1],
                        start=(nl == 0),
                        stop=(nl == NL - 1),
                        skip_group_check=True,
                    )
                rz = sm.tile([P, 1], f32, tag="rz")
                nc.vector.reciprocal(rz[:], psu[:, SLOT : SLOT + 1])
                upd_s = sm.tile([P, SLOT], f32, tag="upd_s")
                nc.vector.tensor_scalar_mul(upd_s[:], psu[:, 0:SLOT], rz[:])

                # reorder upd [hs', d] -> updT [d, s] (per-head transpose)
                updt = slp.tile([P, KC, S], f32r, tag="updt")
                for hh in range(H):
                    pt = pst([DH, S])
                    bp0 = hh * 32
                    nc.tensor.transpose(
                        pt[:],
                        upd_s[bp0 : bp0 + S, ts(hh, DH)],
                        ident[bp0 : bp0 + S, bp0 : bp0 + S],
                        tile_position=(bp0, 0),
                    )
                    nc.scalar.activation(
                        updt[(hh % 2) * 64 : (hh % 2) * 64 + 64, hh // 2, :],
                        pt[:],
                        AF.Copy,
                    )

                # GRU
                hgs = slp.tile([P, GC, S], f32, tag="hgs")
                for gj in range(GC):
                    ph = pst([P, S])
                    for kc in range(KC):
                        nc.tensor.matmul(
                            ph[:],
                            W["wht"][:, kc, ts(gj, P)],
                            slots[:, kc, :],
                            start=(kc == 0),
                            stop=(kc == KC - 1),
                        )
                    nc.scalar.activation(hgs[:, gj, :], ph[:], AF.Copy)
                rzsb = slp.tile([P, 4, S], f32, tag="rzsb")
                nsb = slp.tile([P, KC, S], f32, tag="nsb")
                pxn = []
                for gj in range(GC):
                    px = pst([P, S])
                    for kc in range(KC):
                        nc.tensor.matmul(
                            px[:],
                            W["wit"][:, kc, ts(gj, P)],
                            updt[:, kc, :],
                            start=(kc == 0),
                            stop=(kc == KC - 1),
                        )
                    if gj < 4:
                        tg = slp.tile([P, S], f32, tag="tg")
                        nc.vector.tensor_add(tg[:], px[:], hgs[:, gj, :])
                        nc.scalar.activation(
                            rzsb[:, gj, :], tg[:], AF.Sigmoid,
                            bias=W["brz"][:, gj : gj + 1],
                        )
                    else:
                        pxn.append(px)
                for nj in range(KC):
                    px = pxn[nj]
                    t1 = slp.tile([P, S], f32, tag="t1n")
                    nc.vector.tensor_scalar(
                        t1[:], hgs[:, 4 + nj, :], W["bhn"][:, nj : nj + 1],
                        None, AluOpType.add,
                    )
                    nc.vector.tensor_mul(t1[:], rzsb[:, nj, :], t1[:])
                    nc.vector.tensor_add(t1[:], t1[:], px[:])
                    nc.scalar.activation(
                        nsb[:, nj, :], t1[:], AF.Tanh,
                        bias=W["bin"][:, nj : nj + 1],
                    )
                slots2 = slp.tile([P, KC, S], f32r, tag="slots2")
                for kc in range(KC):
                    dd = slp.tile([P, S], f32, tag="dd")
                    nc.vector.tensor_sub(dd[:], slots[:, kc, :], nsb[:, kc, :])
                    nc.vector.tensor_mul(dd[:], rzsb[:, 2 + kc, :], dd[:])
                    nc.vector.tensor_add(slots2[:, kc, :], nsb[:, kc, :], dd[:])

                # slot MLP with ln_mlp folded + residual
                s1m, invbm = slot_stats_row(slots2)
                hm = slp.tile([P, MC_MLP, S], f32r, tag="hm")
                for j in range(MC_MLP):
                    pz = pst([P, S])
                    for kc in range(KC):
                        nc.tensor.matmul(
                            pz[:],
                            W["m1t"][:, kc, ts(j, P)],
                            slots2[:, kc, :],
                            start=(kc == 0),
                            stop=False,
                        )
                    nc.tensor.matmul(
                        pz[:], W["r1m"][:, ts(j, P)], s1m[:],
                        start=False, stop=True,
                    )
                    tz = slp.tile([P, S], f32, tag="tz")
                    nc.vector.tensor_tensor(tz[:], pz[:], invbm[:], AluOpType.mult)
                    nc.scalar.activation(
                        hm[:, j, :], tz[:], AF.Relu, bias=W["c1m"][:, j : j + 1]
                    )
                slots3 = slp.tile([P, KC, S], f32r, tag="slots")
                for mc in range(KC):
                    p2 = pst([P, S])
                    for j in range(MC_MLP):
                        nc.tensor.matmul(
                            p2[:],
                            W["m2t"][:, j, ts(mc, P)],
                            hm[:, j, :],
                            start=(j == 0),
                            stop=(j == MC_MLP - 1),
                        )
                    tr = slp.tile([P, S], f32, tag="tr")
                    nc.vector.tensor_scalar(
                        tr[:], p2[:], W["b2m"][:, mc : mc + 1], None, AluOpType.add
                    )
                    nc.vector.tensor_add(slots3[:, mc, :], tr[:], slots2[:, mc, :])
                slots = slots3

            # ---------- output head: ln_out folded into out_w, ST layout ----
            # row stats (ones-matrix trick), then a DVE 32x32 transpose turns
            # the [1,S] rows into [S,1] per-partition columns for the ST evac.
            pr1 = pst([P, S])
            for kc in range(KC):
                nc.tensor.matmul(
                    pr1[:], ones_r[:], slots[:, kc, :],
                    start=(kc == 0), stop=(kc == KC - 1),
                )
            pr2 = pst([P, S])
            for kc in range(KC):
                sq = slp.tile([P, S], f32r, tag="sq24")
                nc.scalar.activation(sq[:], slots[:, kc, :], AF.Square)
                nc.tensor.matmul(
                    pr2[:], ones_r[:], sq[:],
                    start=(kc == 0), stop=(kc == KC - 1),
                )
            sqm = slp.tile([P, S], f32, tag="sqm24")
            nc.scalar.activation(sqm[:], pr1[:], AF.Square, scale=1.0 / 16.0)
            nc.vector.tensor_tensor(sqm[:], pr2[:], sqm[:], AluOpType.subtract)
            sdh = slp.tile([P, S], f32, tag="sd24")
            nc.scalar.activation(
                sdh[:], sqm[:], AF.Sqrt, bias=eps_col[:], scale=1.0 / C
            )
            invh = slp.tile([P, S], f32, tag="invb24")
            nc.vector.reciprocal(invh[:], sdh[:])
            mrow = slp.tile([1, S], f32, tag="mrow")
            nc.scalar.activation(mrow[:], pr1[0:1, :], AF.Copy, scale=-1.0 / C)
            # [1,S] inv row -> [S,1] column via a 32x32 DVE block transpose
            scr = slp.tile([32, 32], f32, tag="scr")
            nc.vector.memset(scr[:], 0.0)
            nc.vector.tensor_copy(scr[0:1, 0:S], invh[0:1, :])
            tcol = slp.tile([32, 32], f32, tag="tcol")
            nc.vector.transpose(tcol[:], scr[:])

            po = pst([S, OUT])
            for kc in range(KC):
                nc.tensor.matmul(
                    po[:], slots[:, kc, :].bitcast(f32), W["wot"][:, kc, :],
                    start=(kc == 0), stop=False,
                )
            nc.tensor.matmul(
                po[:], mrow[:], W["ro"][:], start=False, stop=True
            )
            osb = sm.tile([S, OUT], f32, tag="osb")
            nc.vector.tensor_scalar_mul(osb[:], po[:], tcol[0:S, 0:1])
            nc.vector.tensor_add(osb[:], osb[:], coutb[:])
            nc.sync.dma_start(out_d[e], osb[:])

    nc.compile()
    return nc


def _host_prepack(i):
    """Fold LayerNorm affine params into weights, precompute pos embedding."""
    g = lambda k: np.asarray(i[k], np.float32)
    coords = (np.arange(RES, dtype=np.float32) + 0.5) / RES
    gx = np.broadcast_to(coords[None, :], (RES, RES))
    gy = np.broadcast_to(coords[:, None], (RES, RES))
    pe = np.stack([gx, gy, 1.0 - gx, 1.0 - gy], 0).astype(np.float32)
    pos = np.einsum("co,chw->ohw", g("pos_w"), pe).astype(np.float32)
    pos = pos + g("pos_b")[:, None, None]
    x = g("inputs") + pos[None]  # [B, C, RES, RES]
    xin = np.ascontiguousarray(x.reshape(B, KC, P, N))

    def kmaj(w):
        K, M = w.shape
        return np.ascontiguousarray(w.reshape(K // P, P, M).transpose(1, 0, 2))

    def cols(v):
        M = v.shape[0]
        return np.ascontiguousarray(v.reshape(M // P, P).T)

    sh = {}

    def fold(wname, gk, bk, bias=None, scale=1.0):
        w = g(wname)
        wf = (g(gk)[:, None] * w * scale).astype(np.float32)
        rk = (-(wf.sum(0)) / C).reshape(1, -1).astype(np.float32)
        cc = (g(bk) @ w) * scale
        if bias is not None:
            cc = cc + g(bias)
        return kmaj(wf), rk, cols(cc.astype(np.float32))

    sh["w1t"], sh["r1k"], sh["c1c"] = fold(
        "mlp_in_w1", "ln_in_g", "ln_in_b", "mlp_in_b1"
    )
    sh["w2t"] = kmaj(g("mlp_in_w2"))
    sh["b2c"] = cols(g("mlp_in_b2"))
    kscale = float(SLOT) ** -0.5
    sh["wkt"], sh["rkk"], sh["ckc"] = fold("Wk", "ln_inp_g", "ln_inp_b", scale=kscale)
    sh["wvt"], sh["rvk"], sh["cvc"] = fold("Wv", "ln_inp_g", "ln_inp_b")
    sh["wqt"], sh["rqk"], sh["cqc"] = fold("Wq", "ln_slot_g", "ln_slot_b")
    sh["wit"] = kmaj(g("gru_wi"))
    sh["wht"] = kmaj(g("gru_wh"))
    bsum = g("gru_bi") + g("gru_bh")
    sh["brz"] = cols(bsum[0 : 2 * SLOT])
    sh["bhn"] = cols(g("gru_bh")[2 * SLOT :])
    sh["bin"] = cols(g("gru_bi")[2 * SLOT :])
    sh["m1t"], sh["r1m"], sh["c1m"] = fold("mlp_w1", "ln_mlp_g", "ln_mlp_b", "mlp_b1")
    sh["m2t"] = kmaj(g("mlp_w2"))
    sh["b2m"] = cols(g("mlp_b2"))
    wo = g("out_w")
    wof = (g("ln_out_g")[:, None] * wo).astype(np.float32)
    sh["wot"] = kmaj(wof)
    sh["ro"] = wof.sum(0).reshape(1, OUT).astype(np.float32)
    sh["co"] = (g("ln_out_b") @ wo + g("out_b")).reshape(1, OUT).astype(np.float32)
    mu = np.asarray(i["slot_mu"], np.float32)[0]  # [S, SLOT]
    sh["smu"] = np.ascontiguousarray(mu.T.reshape(KC, P, S).transpose(1, 0, 2))
    return sh, xin


_NC_CACHE = {}
LAST_RESULTS = None


def _get_nc():
    if "nc" not in _NC_CACHE:
        _NC_CACHE["nc"] = _build_program(BP)
    return _NC_CACHE["nc"]


def kernel(**inputs):
    global LAST_RESULTS
    nc = _get_nc()
    sh, xin = _host_prepack(inputs)
    in_maps = []
    for c in range(NCORES):
        m = dict(sh)
        m["xin"] = np.ascontiguousarray(xin[c * BP : (c + 1) * BP])
        in_maps.append(m)
    res = bass_utils.run_bass_kernel_spmd(
        nc, in_maps, core_ids=list(range(NCORES))
    )
    LAST_RESULTS = res
    out = np.concatenate([res.results[c]["out"] for c in range(NCORES)], 0)
    return out.astype(np.float32)

